# revision 1
# baseline (speedup 1.0000x reference)
"""Trainium2 Bass kernel for nn_DifferentiableFDN.

Math: the module is linear in x, so
    out[b,t] = sum_j w_j * y_j[b,t],   w = (H^T alpha + beta)/16,
    y_j = first-order IIR of x with decay a_j.

Blocked-scan scheme (chunk length L=128, NCH=375 chunks per batch row).
The host pre-transposes x into XT[b] = (t=128, c=375) and un-transposes the
output, so the device kernel is just 3 matmuls per batch row (stationary
weights, 375-wide moving operands) plus a 375-step DVE scan:
  - e  = P^T  @ XT   (16 x 375)   chunk-end state contributions
  - S  : tensor_tensor_scan over chunks, S[c] = a_j^L S[c-1] + e[c],
         written directly into the shifted position ssh[c] = S[c-1]
  - z  = MT^T @ XT   (128 x 375)  local Toeplitz part  (PSUM, start)
  - z += Wc^T @ ssh  (128 x 375)  rank-16 carry correction (PSUM, stop)
  out[b, c*128+tp] = z[tp, c]

Sharding: pure data-parallel, 4 batch rows per core x 8 cores.
"""
import numpy as np

B, T = 32, 48000
D = 16
NCORES = 8
BL = B // NCORES            # 4 batch rows per core
L = 128                     # chunk length
NCH = T // L                # 375 chunks per batch row

_CACHE = {}


def _mirror_f32_params(log_kappa, alpha_raw, beta_raw, H):
    """Reference param math, f64 internally, rounded through f32 where the
    reference's f32 pipeline rounds."""
    sig = 1.0 / (1.0 + np.exp(-log_kappa.astype(np.float64)))
    sig32 = sig.astype(np.float32)
    kappa = (np.float32(1.0) + sig32 * np.float32(799.0)).astype(np.float32)
    inv = (np.float32(-1.0) / kappa).astype(np.float32)
    decays = np.exp(inv.astype(np.float64)).astype(np.float32)
    decays = np.clip(decays, 0.0, 0.9999).astype(np.float64)
    alpha = (1.0 / (1.0 + np.exp(-alpha_raw.astype(np.float64))))
    beta = (1.0 / (1.0 + np.exp(-beta_raw.astype(np.float64))))
    alpha = alpha.astype(np.float32).astype(np.float64)
    beta = beta.astype(np.float32).astype(np.float64)
    w = (H.astype(np.float64).T @ alpha + beta) / np.float64(D)
    return decays, w


def _tables(decays, w):
    delta = np.arange(L)
    pows = decays[None, :] ** delta[:, None]                   # [L, D] a_j^d
    h = pows @ w                                               # h[d]
    MT = np.zeros((L, L))
    for t in range(L):
        MT[t, t:] = h[: L - t]                                 # MT[t,tp]=h[tp-t]
    P = decays[None, :] ** (L - 1 - delta[:, None])            # [L, D]
    Wc = w[:, None] * decays[:, None] ** (delta[None, :] + 1)  # [D, L]
    ml = np.tile((decays ** L)[:, None], (1, NCH - 1))         # [D, NCH-1]
    f = np.float32
    # pack constants: c1 = [MT | P] (128 x 144), c2 = [Wc | ml] (16 x 502)
    c1 = np.concatenate([MT.astype(f), P.astype(f)], axis=1)
    c2 = np.concatenate([Wc.astype(f), ml.astype(f)], axis=1)
    return np.ascontiguousarray(c1), np.ascontiguousarray(c2)


def _body(tc, o_ap, x_ap, c1_ap, c2_ap):
    from concourse import mybir
    from contextlib import ExitStack

    nc = tc.nc
    f32 = mybir.dt.float32

    bf16 = mybir.dt.bfloat16

    with ExitStack() as ctx:
        const = ctx.enter_context(tc.tile_pool(name="const", bufs=1))
        xtp = ctx.enter_context(tc.tile_pool(name="xt", bufs=1))
        sshp = ctx.enter_context(tc.tile_pool(name="sshp", bufs=1))
        stgp = ctx.enter_context(tc.tile_pool(name="stg", bufs=2))
        warmp = ctx.enter_context(tc.tile_pool(name="warm", bufs=1))
        epp = ctx.enter_context(tc.tile_pool(name="e_ps", bufs=2, space="PSUM"))
        zpp = ctx.enter_context(tc.tile_pool(name="z_ps", bufs=3, space="PSUM"))
        wpp = ctx.enter_context(tc.tile_pool(name="w_ps", bufs=1, space="PSUM"))

        # PE warm-up: HAM throttles the PE to K=4/8 until ~4us of sustained
        # activity; fill the DMA-wait window with dummy bf16 matmuls so the
        # real fp32 work runs at full clock. Inputs are broadcast views of
        # the framework's preloaded const tiles, which are initialized ~1.4us
        # earlier than a kernel-issued memset could land.
        warm_w = nc.const_aps.tensor(1.0, (L, L), bf16)
        warm_x = nc.const_aps.tensor(1.0, (L, 512), bf16)
        w_ps = wpp.tile([L, 512], f32, tag="wps")
        for _ in range(7):
            nc.tensor.matmul(w_ps[:, :], lhsT=warm_w, rhs=warm_x,
                             start=True, stop=True)

        c1 = const.tile([L, 144], f32, tag="c1")
        nc.sync.dma_start(c1[:, :], c1_ap[:, :])
        c2 = const.tile([D, 502], f32, tag="c2")
        nc.scalar.dma_start(c2[:, :], c2_ap[:, :])
        mt_sb, p_sb = c1[:, 0:128], c1[:, 128:144]
        wc_sb, ml_sb = c2[:, 0:128], c2[:, 128:502]

        xt = [xtp.tile([L, NCH], f32, tag=f"xt{b}", name=f"xt{b}")
              for b in range(BL)]
        ssh = [sshp.tile([D, NCH], f32, tag=f"ssh{b}", name=f"ssh{b}")
               for b in range(BL)]

        for b in range(BL):
            eng = nc.sync if b % 2 == 0 else nc.scalar
            eng.dma_start(xt[b][:, :], x_ap[b * L:(b + 1) * L, :])

        # chunk-end states first: E matmuls feed the DVE scans, which run
        # behind the remaining E's so the corr matmuls never stall on them
        for b in range(BL):
            e_ps = epp.tile([D, NCH], f32, tag="e")
            nc.tensor.matmul(e_ps[:, :], lhsT=p_sb, rhs=xt[b][:, :],
                             start=True, stop=True)
            nc.vector.tensor_tensor_scan(
                ssh[b][:, 1:NCH], data0=ml_sb, data1=e_ps[:, 0:NCH - 1],
                initial=0.0, op0=mybir.AluOpType.mult, op1=mybir.AluOpType.add)
            nc.vector.memset(ssh[b][:, 0:1], 0.0)

        for b in range(BL):
            z_ps = zpp.tile([L, NCH], f32, tag="z")
            nc.tensor.matmul(z_ps[:, :], lhsT=mt_sb, rhs=xt[b][:, :],
                             start=True, stop=False, skip_group_check=True)
            nc.tensor.matmul(z_ps[:, :], lhsT=wc_sb, rhs=ssh[b][:, :],
                             start=False, stop=True, skip_group_check=True)
            stg = stgp.tile([L, NCH], f32, tag="stg")
            nc.vector.tensor_copy(stg[:, :], z_ps[:, :])
            eng = nc.sync if b % 2 == 0 else nc.scalar
            eng.dma_start(o_ap[:, b * NCH:(b + 1) * NCH], stg[:, :])


def _build(num_devices=NCORES):
    import concourse.tile as tile
    from concourse import bacc, mybir

    f32 = mybir.dt.float32
    nc = bacc.Bacc("TRN2", target_bir_lowering=False, debug=False,
                   num_devices=num_devices)
    x_ap = nc.dram_tensor("x", [BL * L, NCH], f32, kind="ExternalInput").ap()
    c1_ap = nc.dram_tensor("c1", [L, 144], f32, kind="ExternalInput").ap()
    c2_ap = nc.dram_tensor("c2", [D, 502], f32, kind="ExternalInput").ap()
    o_ap = nc.dram_tensor("out", [L, BL * NCH], f32, kind="ExternalOutput").ap()

    with tile.TileContext(nc) as tc:
        _body(tc, o_ap, x_ap, c1_ap, c2_ap)
    nc.compile()
    return nc


def _in_maps(x, log_kappa, alpha_raw, beta_raw, H):
    decays, w = _mirror_f32_params(np.asarray(log_kappa), np.asarray(alpha_raw),
                                   np.asarray(beta_raw), np.asarray(H))
    c1, c2 = _tables(decays, w)
    x = np.ascontiguousarray(np.asarray(x), dtype=np.float32)
    # host pre-transpose: (B, T) -> per-core (BL*L, NCH) chunk-transposed
    xt_all = x.reshape(B, NCH, L).transpose(0, 2, 1)  # (B, L, NCH)
    maps = []
    for c in range(NCORES):
        xs = np.ascontiguousarray(xt_all[c * BL:(c + 1) * BL]).reshape(BL * L, NCH)
        maps.append({"x": xs, "c1": c1, "c2": c2})
    return maps


def _gather(results):
    # out dram per core: (L, BL*NCH) = [tp, (b, c)] -> (BL, T)
    outs = []
    for c in range(NCORES):
        arr = results[c]["out"].reshape(L, BL, NCH)
        outs.append(arr.transpose(1, 2, 0).reshape(BL, T))  # out[b, c*L+tp]
    return np.concatenate(outs, axis=0)


def kernel(x, log_kappa, alpha_raw, beta_raw, H):
    from concourse import bass_utils

    if "nc" not in _CACHE:
        _CACHE["nc"] = _build()
    nc = _CACHE["nc"]
    maps = _in_maps(x, log_kappa, alpha_raw, beta_raw, H)
    res = bass_utils.run_bass_kernel_spmd(nc, maps, core_ids=list(range(NCORES)))
    return _gather(res.results).astype(np.float32)



# revision 11
# speedup vs baseline: 1.2564x; 1.2564x over previous
"""Trainium2 Bass kernel for nn_DifferentiableFDN.

Math: the module is linear in x, so
    out[b,t] = sum_j w_j * y_j[b,t],   w = (H^T alpha + beta)/16,
    y_j = first-order IIR of x with decay a_j.

Blocked-scan scheme (chunk length L=128, NCH=375 chunks per batch row).
The host pre-transposes x into XT[b] = (t=128, c=375) and un-transposes the
output. All matmul operands are bf16 (PSUM accumulates fp32; the 2e-2 rel-err
gate leaves ~50x headroom); the chunk-carry scan state stays fp32 inside the
DVE. Per batch row:
  - e  = P^T  @ XT   (16 x 375)   chunk-end state contributions, written at
         partition offset 16b into one stacked PSUM tile (64 x 375)
  - S  : ONE tensor_tensor_scan over the stacked tile, S[c] = a_j^L S[c-1]+e[c],
         written bf16 into the shifted position ssh[c] = S[c-1]
  - z  = MT^T @ XT   (128 x 375)  local Toeplitz part  (PSUM, start)
  - z += Wc^T @ ssh  (128 x 375)  rank-16 carry correction (PSUM, stop)
  out[b, c*128+tp] = z[tp, c], staged to SBUF as bf16, host converts to f32.

Matmuls are grouped by stationary weights (P x4, MT x4, Wc x4) so the PE can
keep weights loaded. No warm-up matmuls: the kernel is shorter than the HAM
ramp, so the PE runs at the throttled clock either way, and every extra
Tensor-queue instruction costs ~115ns in the framework's end-of-kernel
semaphore clear (the dominant fixed tail).

Sharding: pure data-parallel, 4 batch rows per core x 8 cores.
"""
import numpy as np
import ml_dtypes

B, T = 32, 48000
D = 16
NCORES = 8
BL = B // NCORES            # 4 batch rows per core
L = 128                     # chunk length
NCH = T // L                # 375 chunks per batch row

_CACHE = {}


def _mirror_f32_params(log_kappa, alpha_raw, beta_raw, H):
    """Reference param math, f64 internally, rounded through f32 where the
    reference's f32 pipeline rounds."""
    sig = 1.0 / (1.0 + np.exp(-log_kappa.astype(np.float64)))
    sig32 = sig.astype(np.float32)
    kappa = (np.float32(1.0) + sig32 * np.float32(799.0)).astype(np.float32)
    inv = (np.float32(-1.0) / kappa).astype(np.float32)
    decays = np.exp(inv.astype(np.float64)).astype(np.float32)
    decays = np.clip(decays, 0.0, 0.9999).astype(np.float64)
    alpha = (1.0 / (1.0 + np.exp(-alpha_raw.astype(np.float64))))
    beta = (1.0 / (1.0 + np.exp(-beta_raw.astype(np.float64))))
    alpha = alpha.astype(np.float32).astype(np.float64)
    beta = beta.astype(np.float32).astype(np.float64)
    w = (H.astype(np.float64).T @ alpha + beta) / np.float64(D)
    return decays, w


def _tables(decays, w):
    delta = np.arange(L)
    pows = decays[None, :] ** delta[:, None]                   # [L, D] a_j^d
    h = pows @ w                                               # h[d]
    MT = np.zeros((L, L))
    for t in range(L):
        MT[t, t:] = h[: L - t]                                 # MT[t,tp]=h[tp-t]
    P = decays[None, :] ** (L - 1 - delta[:, None])            # [L, D]
    Wc = w[:, None] * decays[:, None] ** (delta[None, :] + 1)  # [D, L]
    bf = ml_dtypes.bfloat16
    # c1 = [MT | P] (128 x 144) bf16.
    # The 4 batch rows' chunk-end states live at PSUM partition offsets
    # 0/32/64/96 (the only legal PE output tile positions), so the corr
    # weights and scan multiplier are replicated at those offsets:
    # c2 = Wc at partitions 32b..32b+16 (128 x 128) bf16; mlc likewise
    # (128 x 1) f32 (scan multiplier, broadcast along columns on device).
    c1 = np.concatenate([MT, P], axis=1).astype(bf)
    c2 = np.zeros((L, L), dtype=bf)
    mlc = np.zeros((L, 1), dtype=np.float32)
    for b in range(BL):
        c2[32 * b:32 * b + D, :] = Wc.astype(bf)
        mlc[32 * b:32 * b + D, 0] = (decays ** L).astype(np.float32)
    return (np.ascontiguousarray(c1), np.ascontiguousarray(c2),
            np.ascontiguousarray(mlc))


def _body(tc, o_ap, x_ap, c1_ap, c2_ap, ml_ap):
    from concourse import mybir
    from contextlib import ExitStack

    nc = tc.nc
    f32 = mybir.dt.float32
    bf16 = mybir.dt.bfloat16

    with ExitStack() as ctx:
        const = ctx.enter_context(tc.tile_pool(name="const", bufs=1))
        xtp = ctx.enter_context(tc.tile_pool(name="xt", bufs=1))
        sshp = ctx.enter_context(tc.tile_pool(name="sshp", bufs=1))
        stgp = ctx.enter_context(tc.tile_pool(name="stg", bufs=1))
        epp = ctx.enter_context(tc.tile_pool(name="e_ps", bufs=1, space="PSUM"))
        zpp = ctx.enter_context(tc.tile_pool(name="z_ps", bufs=1, space="PSUM"))

        c1 = const.tile([L, 144], bf16, tag="c1")
        c2 = const.tile([L, L], bf16, tag="c2")
        mlc = const.tile([L, 1], f32, tag="mlc")
        xt = [xtp.tile([L, NCH], bf16, tag=f"xt{b}", name=f"xt{b}")
              for b in range(BL)]
        ssh = sshp.tile([L, NCH], bf16, tag="ssh")
        e_all = epp.tile([L, NCH], f32, tag="e")

        # input DMAs: two HW queues (sync=SP, scalar=Activation).  Small
        # consts lead their queue so the first E matmul is gated only on xt0.
        nc.sync.dma_start(c1[:, :], c1_ap[:, :])
        nc.scalar.dma_start(c2[:, :], c2_ap[:, :])
        nc.scalar.dma_start(mlc[:, :], ml_ap[:, :])
        nc.sync.dma_start(xt[0][:, :], x_ap[0 * L:1 * L, :])
        nc.scalar.dma_start(xt[1][:, :], x_ap[1 * L:2 * L, :])
        nc.sync.dma_start(xt[2][:, :], x_ap[2 * L:3 * L, :])
        nc.scalar.dma_start(xt[3][:, :], x_ap[3 * L:4 * L, :])

        # scan writes cols 1..NCH-1; col 0 is the zero initial state
        nc.gpsimd.memset(ssh[:, 0:1], 0.0)

        mt_sb, p_sb = c1[:, 0:128], c1[:, 128:144]

        # chunk-end states: 4 matmuls, same stationary P, partition-offset
        # writes (tile positions 0/32/64/96) into one stacked PSUM tile
        for b in range(BL):
            nc.tensor.matmul(e_all[32 * b:32 * b + D, :], lhsT=p_sb,
                             rhs=xt[b][:, :], start=True, stop=True,
                             skip_group_check=True, tile_position=(0, 32 * b))

        # ONE scan for all 4 batch rows (DVE cost is per-column, not
        # per-partition); fp32 state internally, bf16 output. The gap
        # partitions carry garbage that nothing reads.
        nc.vector.tensor_tensor_scan(
            ssh[:, 1:NCH], data0=mlc[:, 0:1].broadcast_to((L, NCH - 1)),
            data1=e_all[:, 0:NCH - 1],
            initial=0.0, op0=mybir.AluOpType.mult, op1=mybir.AluOpType.add)

        z = [zpp.tile([L, NCH], f32, tag=f"z{b}", name=f"z{b}")
             for b in range(BL)]
        for b in range(BL):
            nc.tensor.matmul(z[b][:, :], lhsT=mt_sb, rhs=xt[b][:, :],
                             start=True, stop=False, skip_group_check=True)
        for b in range(BL):
            nc.tensor.matmul(z[b][:, :], lhsT=c2[32 * b:32 * b + D, :],
                             rhs=ssh[32 * b:32 * b + D, :],
                             start=False, stop=True, skip_group_check=True,
                             tile_position=(32 * b, 0))

        copy_eng = [nc.vector, nc.scalar, nc.vector, nc.scalar]
        dma_eng = [nc.sync, nc.scalar, nc.sync, nc.scalar]
        for b in range(BL):
            stg = stgp.tile([L, NCH], bf16, tag=f"stg{b}")
            if copy_eng[b] is nc.scalar:
                copy_eng[b].copy(stg[:, :], z[b][:, :])
            else:
                copy_eng[b].tensor_copy(stg[:, :], z[b][:, :])
            dma_eng[b].dma_start(o_ap[b * L:(b + 1) * L, :], stg[:, :])


def _build(num_devices=NCORES):
    import concourse.tile as tile
    from concourse import bacc, mybir

    f32 = mybir.dt.float32
    bf16 = mybir.dt.bfloat16
    nc = bacc.Bacc("TRN2", target_bir_lowering=False, debug=False,
                   num_devices=num_devices)
    x_ap = nc.dram_tensor("x", [BL * L, NCH], bf16, kind="ExternalInput").ap()
    c1_ap = nc.dram_tensor("c1", [L, 144], bf16, kind="ExternalInput").ap()
    c2_ap = nc.dram_tensor("c2", [L, L], bf16, kind="ExternalInput").ap()
    ml_ap = nc.dram_tensor("mlc", [L, 1], f32, kind="ExternalInput").ap()
    o_ap = nc.dram_tensor("out", [BL * L, NCH], bf16, kind="ExternalOutput").ap()

    with tile.TileContext(nc) as tc:
        _body(tc, o_ap, x_ap, c1_ap, c2_ap, ml_ap)
    nc.compile()
    return nc


def _in_maps(x, log_kappa, alpha_raw, beta_raw, H):
    decays, w = _mirror_f32_params(np.asarray(log_kappa), np.asarray(alpha_raw),
                                   np.asarray(beta_raw), np.asarray(H))
    c1, c2, mlc = _tables(decays, w)
    bf = ml_dtypes.bfloat16
    x = np.asarray(x, dtype=np.float32)
    # host pre-transpose: (B, T) -> per-core (BL*L, NCH) chunk-transposed, bf16
    xt_all = x.reshape(B, NCH, L).transpose(0, 2, 1).astype(bf)  # (B, L, NCH)
    maps = []
    for c in range(NCORES):
        xs = np.ascontiguousarray(xt_all[c * BL:(c + 1) * BL]).reshape(BL * L, NCH)
        maps.append({"x": xs, "c1": c1, "c2": c2, "mlc": mlc})
    return maps


def _gather(results):
    # out dram per core: (BL*L, NCH) = [(b, tp), c] -> (BL, T), t = c*L + tp
    outs = []
    for c in range(NCORES):
        arr = np.asarray(results[c]["out"]).reshape(BL, L, NCH)
        outs.append(arr.transpose(0, 2, 1).reshape(BL, T))
    return np.concatenate(outs, axis=0).astype(np.float32)


def kernel(x, log_kappa, alpha_raw, beta_raw, H):
    from concourse import bass_utils

    if "nc" not in _CACHE:
        _CACHE["nc"] = _build()
    nc = _CACHE["nc"]
    maps = _in_maps(x, log_kappa, alpha_raw, beta_raw, H)
    res = bass_utils.run_bass_kernel_spmd(nc, maps, core_ids=list(range(NCORES)))
    return _gather(res.results)


# revision 17
# speedup vs baseline: 1.4438x; 1.1492x over previous
"""Trainium2 Bass kernel for nn_DifferentiableFDN.

Math: the module is linear in x, so
    out[b,t] = sum_j w_j * y_j[b,t],   w = (H^T alpha + beta)/16,
    y_j = first-order IIR of x with decay a_j.

Blocked-scan scheme (chunk length L=128, NCH=375 chunks per batch row).
The host pre-transposes x into XT[b] = (t=128, c=375) and un-transposes the
output. All matmul operands are bf16 (PSUM accumulates fp32; the 2e-2 rel-err
gate leaves ~50x headroom); the chunk-carry scan state stays fp32 inside the
DVE. Per batch row:
  - e  = P^T  @ XT   (16 x 375)   chunk-end state contributions, written at
         partition offset 16b into one stacked PSUM tile (64 x 375)
  - S  : ONE tensor_tensor_scan over the stacked tile, S[c] = a_j^L S[c-1]+e[c],
         written bf16 into the shifted position ssh[c] = S[c-1]
  - z  = MT^T @ XT   (128 x 375)  local Toeplitz part  (PSUM, start)
  - z += Wc^T @ ssh  (128 x 375)  rank-16 carry correction (PSUM, stop)
  out[b, c*128+tp] = z[tp, c], staged to SBUF as bf16, host converts to f32.

Matmuls are grouped by stationary weights (P x4, MT x4, Wc x4) so the PE can
keep weights loaded. No warm-up matmuls: the kernel is shorter than the HAM
ramp, so the PE runs at the throttled clock either way, and every extra
Tensor-queue instruction costs ~115ns in the framework's end-of-kernel
semaphore clear (the dominant fixed tail).

Sharding: pure data-parallel, 4 batch rows per core x 8 cores.
"""
import numpy as np
import ml_dtypes

B, T = 32, 48000
D = 16
NCORES = 8
BL = B // NCORES            # 4 batch rows per core
L = 128                     # chunk length
NCH = T // L                # 375 chunks per batch row

_CACHE = {}


def _mirror_f32_params(log_kappa, alpha_raw, beta_raw, H):
    """Reference param math, f64 internally, rounded through f32 where the
    reference's f32 pipeline rounds."""
    sig = 1.0 / (1.0 + np.exp(-log_kappa.astype(np.float64)))
    sig32 = sig.astype(np.float32)
    kappa = (np.float32(1.0) + sig32 * np.float32(799.0)).astype(np.float32)
    inv = (np.float32(-1.0) / kappa).astype(np.float32)
    decays = np.exp(inv.astype(np.float64)).astype(np.float32)
    decays = np.clip(decays, 0.0, 0.9999).astype(np.float64)
    alpha = (1.0 / (1.0 + np.exp(-alpha_raw.astype(np.float64))))
    beta = (1.0 / (1.0 + np.exp(-beta_raw.astype(np.float64))))
    alpha = alpha.astype(np.float32).astype(np.float64)
    beta = beta.astype(np.float32).astype(np.float64)
    w = (H.astype(np.float64).T @ alpha + beta) / np.float64(D)
    return decays, w


def _tables(decays, w):
    delta = np.arange(L)
    pows = decays[None, :] ** delta[:, None]                   # [L, D] a_j^d
    h = pows @ w                                               # h[d]
    MT = np.zeros((L, L))
    for t in range(L):
        MT[t, t:] = h[: L - t]                                 # MT[t,tp]=h[tp-t]
    P = decays[None, :] ** (L - 1 - delta[:, None])            # [L, D]
    Wc = w[:, None] * decays[:, None] ** (delta[None, :] + 1)  # [D, L]
    bf = ml_dtypes.bfloat16
    # c1 = [MT | P] (128 x 144) bf16.
    # The 4 batch rows' chunk-end states live at PSUM partition offsets
    # 0/32/64/96 (the only legal PE output tile positions), so the corr
    # weights and scan multiplier are replicated at those offsets:
    # c2 = Wc at partitions 32b..32b+16 (128 x 128) bf16; mlc likewise
    # (128 x 1) f32 (scan multiplier, broadcast along columns on device).
    c1 = np.concatenate([MT, P], axis=1).astype(bf)
    c2 = np.zeros((L, L), dtype=bf)
    mlc = np.zeros((L, 1), dtype=np.float32)
    for b in range(BL):
        c2[32 * b:32 * b + D, :] = Wc.astype(bf)
        mlc[32 * b:32 * b + D, 0] = (decays ** L).astype(np.float32)
    return (np.ascontiguousarray(c1), np.ascontiguousarray(c2),
            np.ascontiguousarray(mlc))


def _body(tc, o_ap, x_ap, c1_ap, c2_ap, ml_ap):
    from concourse import mybir
    from contextlib import ExitStack

    nc = tc.nc
    f32 = mybir.dt.float32
    bf16 = mybir.dt.bfloat16

    with ExitStack() as ctx:
        const = ctx.enter_context(tc.tile_pool(name="const", bufs=1))
        xtp = ctx.enter_context(tc.tile_pool(name="xt", bufs=1))
        sshp = ctx.enter_context(tc.tile_pool(name="sshp", bufs=1))
        stgp = ctx.enter_context(tc.tile_pool(name="stg", bufs=1))
        epp = ctx.enter_context(tc.tile_pool(name="e_ps", bufs=1, space="PSUM"))
        zpp = ctx.enter_context(tc.tile_pool(name="z_ps", bufs=1, space="PSUM"))

        c1 = const.tile([L, 144], bf16, tag="c1")
        c2 = const.tile([L, L], bf16, tag="c2")
        mlc = const.tile([L, 1], f32, tag="mlc")
        # batch rows are PAIRED per SBUF tile: 1500B partition lines keep the
        # DMA queues at full rate (750B lines run at ~half throughput)
        xtq = [xtp.tile([L, 2 * NCH], bf16, tag=f"xt{q}", name=f"xt{q}")
               for q in range(2)]
        xt = [xtq[b // 2][:, (b % 2) * NCH:(b % 2 + 1) * NCH] for b in range(BL)]
        ssh = sshp.tile([L, NCH], bf16, tag="ssh")
        e_all = epp.tile([L, NCH], f32, tag="e")

        # input DMAs: two HW queues (sync=SP, scalar=Activation).  Small
        # consts lead their queue so the E matmuls are gated only on x.
        nc.sync.dma_start(mlc[:, :], ml_ap[:, :])
        nc.sync.dma_start(c1[:, :], c1_ap[:, :])
        nc.scalar.dma_start(c2[:, :], c2_ap[:, :])
        nc.sync.dma_start(xtq[0][:, :], x_ap[0:L, :])
        nc.scalar.dma_start(xtq[1][:, :], x_ap[L:2 * L, :])

        # scan writes cols 1..NCH-1; col 0 is the zero initial state
        nc.gpsimd.memset(ssh[:, 0:1], 0.0)

        mt_sb, p_sb = c1[:, 0:128], c1[:, 128:144]

        # chunk-end states: 4 matmuls, same stationary P, partition-offset
        # writes (tile positions 0/32/64/96) into one stacked PSUM tile;
        # disjoint column quadrants let all four run concurrently on the PE
        for b in range(BL):
            nc.tensor.matmul(e_all[32 * b:32 * b + D, :], lhsT=p_sb,
                             rhs=xt[b], start=True, stop=True,
                             skip_group_check=True, tile_position=(0, 32 * b))

        # ONE scan for all 4 batch rows (DVE cost is per-column, not
        # per-partition); fp32 state internally, bf16 output. The gap
        # partitions carry garbage that nothing reads.
        nc.vector.tensor_tensor_scan(
            ssh[:, 1:NCH], data0=mlc[:, 0:1].broadcast_to((L, NCH - 1)),
            data1=e_all[:, 0:NCH - 1],
            initial=0.0, op0=mybir.AluOpType.mult, op1=mybir.AluOpType.add)

        z = [zpp.tile([L, NCH], f32, tag=f"z{b}", name=f"z{b}")
             for b in range(BL)]
        for b in range(BL):
            nc.tensor.matmul(z[b][:, :], lhsT=mt_sb, rhs=xt[b][:, :],
                             start=True, stop=False, skip_group_check=True)
        for b in range(BL):
            nc.tensor.matmul(z[b][:, :], lhsT=c2[32 * b:32 * b + D, :],
                             rhs=ssh[32 * b:32 * b + D, :],
                             start=False, stop=True, skip_group_check=True,
                             tile_position=(32 * b, 0))

        # staging is paired too (1500B lines, 2 output DMAs); within a pair
        # one copy runs on the DVE and one on the Activation engine
        stq = [stgp.tile([L, 2 * NCH], bf16, tag=f"stg{q}", name=f"stg{q}")
               for q in range(2)]
        for b in range(BL):
            dst = stq[b // 2][:, (b % 2) * NCH:(b % 2 + 1) * NCH]
            if b % 2:
                nc.scalar.copy(dst, z[b][:, :])
            else:
                nc.vector.tensor_copy(dst, z[b][:, :])
        nc.sync.dma_start(o_ap[:, 0:2 * NCH], stq[0][:, :])
        nc.scalar.dma_start(o_ap[:, 2 * NCH:4 * NCH], stq[1][:, :])


def _build(num_devices=NCORES):
    import concourse.tile as tile
    from concourse import bacc, mybir

    f32 = mybir.dt.float32
    bf16 = mybir.dt.bfloat16
    nc = bacc.Bacc("TRN2", target_bir_lowering=False, debug=False,
                   num_devices=num_devices)
    # x rows 0..127 = queue 0 (b0|b1 column-paired), rows 128..255 = queue 1
    x_ap = nc.dram_tensor("x", [2 * L, 2 * NCH], bf16, kind="ExternalInput").ap()
    c1_ap = nc.dram_tensor("c1", [L, 144], bf16, kind="ExternalInput").ap()
    c2_ap = nc.dram_tensor("c2", [L, L], bf16, kind="ExternalInput").ap()
    ml_ap = nc.dram_tensor("mlc", [L, 1], f32, kind="ExternalInput").ap()
    # out[tp, b*NCH + c]
    o_ap = nc.dram_tensor("out", [L, BL * NCH], bf16, kind="ExternalOutput").ap()

    with tile.TileContext(nc) as tc:
        _body(tc, o_ap, x_ap, c1_ap, c2_ap, ml_ap)
    nc.compile()
    return nc


def _in_maps(x, log_kappa, alpha_raw, beta_raw, H):
    decays, w = _mirror_f32_params(np.asarray(log_kappa), np.asarray(alpha_raw),
                                   np.asarray(beta_raw), np.asarray(H))
    c1, c2, mlc = _tables(decays, w)
    bf = ml_dtypes.bfloat16
    x = np.asarray(x, dtype=np.float32)
    # host pre-transpose: (B, T) -> per-core (2*L, 2*NCH) with batch rows
    # column-paired per DMA queue, bf16
    xt_all = x.reshape(B, NCH, L).transpose(0, 2, 1).astype(bf)  # (B, L, NCH)
    maps = []
    for c in range(NCORES):
        quad = xt_all[c * BL:(c + 1) * BL]           # (4, L, NCH)
        xs = quad.reshape(2, 2, L, NCH).transpose(0, 2, 1, 3).reshape(
            2 * L, 2 * NCH)                          # row q*L+p, col b*NCH+c
        maps.append({"x": np.ascontiguousarray(xs), "c1": c1, "c2": c2,
                     "mlc": mlc})
    return maps


def _gather(results):
    # out dram per core: (L, BL*NCH) = [tp, (b, c)] -> (BL, T), t = c*L + tp
    outs = []
    for c in range(NCORES):
        arr = np.asarray(results[c]["out"]).reshape(L, BL, NCH)
        outs.append(arr.transpose(1, 2, 0).reshape(BL, T))
    return np.concatenate(outs, axis=0).astype(np.float32)


def kernel(x, log_kappa, alpha_raw, beta_raw, H):
    from concourse import bass_utils

    if "nc" not in _CACHE:
        _CACHE["nc"] = _build()
    nc = _CACHE["nc"]
    maps = _in_maps(x, log_kappa, alpha_raw, beta_raw, H)
    res = bass_utils.run_bass_kernel_spmd(nc, maps, core_ids=list(range(NCORES)))
    return _gather(res.results)
